# revision 8
# baseline (speedup 1.0000x reference)
"""Trainium2 Bass kernel for nn_DimCosSoftmaxModule (8-core SPMD).

Math (exact refactor of the reference):
  k1[n,j,t] = relu(sum_i mem_feat[n,i] wt[i,j,t] + bt[j])                 [200,2048,3]
  k2[n,o,s] = relu(sum_{i,dt} wc[o,i,dt] k1pad[n,i,s+dt-1] + bc[o])      [200,2048,3]
  conv/sp_down fold: cls[b,n] = sum_{i,t} G[b,i,t] k2[n,i,t] + b_sp
      where G[b,i,t] = sum_u feat[b,i,u] V[u,t],  V = shifted copies of w_sp
  out = 30*(cosine(cls, w_cls) - 0.5*onehot(label))

Sharding: tensor-parallel over the 2048 channel dim (256 ch/core).
  step1 column-sharded -> 2 pipelined AllGathers (one per 128-wide jc half)
  -> step2 o-sharded -> partial cls -> ReduceScatter (core c keeps batch
  rows 8c..8c+8) -> row-local CosFace.  Host reassembles the 8 row-shards.

Perf structure (vs the 185us baseline):
  * all input DMAs are contiguous per partition (host pre-layout), split
    across the three DMA rails (SP / ACT HWDGE + Pool SWDGE)
  * feat is stored fp8(e4m3) in DRAM and cast to bf16 during the SWDGE
    DMA itself; all PE math stays bf16 (sim rel-err 1.27e-2 < 2e-2)
  * scale folding keeps everything in fp8/bf16 range: wt*32, wc*32, V*16;
    the cosine normalization cancels the net cls scale (b_sp scaled too)
  * the AllGather is split into two halves pipelined behind step1, and
    step2 consumes the halves in order; G runs after step2 (feat lands
    last), keeping the PE continuously busy (p-state stays hot)
"""
import numpy as np
import ml_dtypes

import concourse.bass as bass
import concourse.bacc as bacc
import concourse.mybir as mybir
import concourse.tile as tile
from concourse import bass_utils
from concourse.masks import make_identity

N_CORES = 8
BS, C, HW = 64, 2048, 196
NM = 200                 # N_MEM == NUM_CLASSES
SH = C // N_CORES        # 256 channels per core
NIT = C // 128           # 16 i-tiles of 128
RB = BS // N_CORES       # 8 batch rows per core after reduce-scatter
S_SCALE, M_MARGIN = 30.0, 0.5
SWT, SWC, SV = 32.0, 32.0, 16.0     # fold scales (cancel in cosine)

BF16 = mybir.dt.bfloat16
F32 = mybir.dt.float32
FP8 = mybir.dt.float8e4
AF = mybir.ActivationFunctionType
ALU = mybir.AluOpType

TRACE = False
TRACE_KW = {}
LAST_RESULT = None
_CACHE = {}

NX = BS * SH          # 16384 G columns per core
NQ = NX // 128        # 128 x-chunks


def build_nc():
    nc = bacc.Bacc("TRN2", target_bir_lowering=False, debug=False, num_devices=N_CORES)

    # per-core external inputs (same shapes on every core, different data)
    MFT = nc.dram_tensor("mft", [128, NIT * NM], BF16, kind="ExternalInput")
    WTS = nc.dram_tensor("wts", [128, 2 * 3 * NIT * 128], BF16, kind="ExternalInput")
    WCT = nc.dram_tensor("wct", [128, NIT * 3 * SH], BF16, kind="ExternalInput")
    FTQ = nc.dram_tensor("ftq", [HW, NX], FP8, kind="ExternalInput")
    VM = nc.dram_tensor("vm", [HW, 3], BF16, kind="ExternalInput")
    BT = nc.dram_tensor("btc", [128, 2], F32, kind="ExternalInput")
    BC = nc.dram_tensor("bcc", [128, 2], F32, kind="ExternalInput")
    WCLS = nc.dram_tensor("wclsT", [NM, NM], F32, kind="ExternalInput")
    BSP = nc.dram_tensor("bsp", [BS, 1], F32, kind="ExternalInput")
    LBL = nc.dram_tensor("lbl", [RB, 1], F32, kind="ExternalInput")
    IOTA = nc.dram_tensor("iota", [RB, NM], F32, kind="ExternalInput")
    Y = nc.dram_tensor("y", [RB, NM], F32, kind="ExternalOutput")

    with tile.TileContext(nc) as tc:
        with (
            tc.tile_pool(name="sbuf", bufs=1) as sbuf,
            tc.tile_pool(name="psum", bufs=1, space="PSUM") as psum,
            tc.tile_pool(name="dram", bufs=1, space="DRAM") as dram,
        ):
            # ---------------- input DMAs: 3 rails, priority order ----------------
            # SP rail: wt halves (step1 weights), then dynamic traffic later.
            wt_sb = sbuf.tile([128, 2, 3, NIT, 128], BF16, tag="wts")
            wtv = WTS.rearrange("p (jc r) -> p jc r", jc=2)
            nc.sync.dma_start(wt_sb[:, 0].rearrange("p t it j -> p (t it j)"), wtv[:, 0])
            nc.sync.dma_start(wt_sb[:, 1].rearrange("p t it j -> p (t it j)"), wtv[:, 1])

            # ACT rail: mem_feat first (step1 rhs), smalls, then step2 weights.
            mf_sb = sbuf.tile([128, NIT, NM], BF16, tag="mf")
            nc.scalar.dma_start(mf_sb.rearrange("p a b -> p (a b)"), MFT[:, :])
            bt_sb = sbuf.tile([128, 2], F32, tag="bt")
            nc.scalar.dma_start(bt_sb[:], BT[:])
            bc_sb = sbuf.tile([128, 2], F32, tag="bc")
            nc.scalar.dma_start(bc_sb[:], BC[:])
            v_sb = sbuf.tile([128, 2, 3], BF16, tag="v")
            nc.scalar.dma_start(v_sb[0:128, 0, :], VM[0:128, :])
            nc.scalar.dma_start(v_sb[0:68, 1, :], VM[128:HW, :])
            wcT_sb = sbuf.tile([128, NIT, 3, SH], BF16, tag="wcT")
            wcv = WCT.rearrange("p (hh r) -> p hh r", hh=2)
            nc.scalar.dma_start(
                wcT_sb[:, 0:8].rearrange("p it t j -> p (it t j)"), wcv[:, 0])
            nc.scalar.dma_start(
                wcT_sb[:, 8:16].rearrange("p it t j -> p (it t j)"), wcv[:, 1])
            wcls_sb = sbuf.tile([128, 2, NM], F32, tag="wcls")
            nc.scalar.dma_start(wcls_sb[0:128, 0, :], WCLS[0:128, :])
            nc.scalar.dma_start(wcls_sb[0:72, 1, :], WCLS[128:NM, :])
            bsp_sb = sbuf.tile([BS, 1], F32, tag="bsp")
            nc.scalar.dma_start(bsp_sb[:], BSP[:])
            lbl_sb = sbuf.tile([RB, 1], F32, tag="lbl")
            nc.scalar.dma_start(lbl_sb[:], LBL[:])
            iota_sb = sbuf.tile([RB, NM], F32, tag="iota")
            nc.scalar.dma_start(iota_sb[:], IOTA[:])

            # POOL rail (SWDGE): feat, fp8 in DRAM cast to bf16 in-flight.
            ft0_sb = sbuf.tile([128, NX], BF16, tag="ft0")
            ft1_sb = sbuf.tile([68, NX], BF16, tag="ft1")
            nc.gpsimd.dma_start(ft0_sb[:], FTQ[0:128, :])
            nc.gpsimd.dma_start(ft1_sb[:], FTQ[128:HW, :])

            # ---------------- constants ----------------
            idn = sbuf.tile([128, 128], F32, tag="idn")
            make_identity(nc, idn[:])
            ones1 = sbuf.tile([1, RB], F32, tag="ones1")
            nc.vector.memset(ones1[:], 1.0)
            onesc = sbuf.tile([128, 1], F32, tag="onesc")
            nc.vector.memset(onesc[:], 1.0)

            # ---------------- step 1 + pipelined AllGather halves ----------------
            # k1'[j,t,n] = relu(SWT * (mem_feat wt + bt)) for this core's 256 j.
            k1_sb = sbuf.tile([128, 2, 3, NM], BF16, tag="k1")
            kb = [dram.tile([128, 3 * NM], BF16, name=f"k1_bounce{h}") for h in range(2)]
            kg = [dram.tile([N_CORES * 128, 3 * NM], BF16, name=f"k1_gath{h}")
                  for h in range(2)]
            for jc in range(2):
                for t in range(3):
                    ps1 = psum.tile([128, NM], F32, tag="ps1", bufs=2,
                                    name=f"ps1_{jc}_{t}")
                    for it in range(NIT):
                        nc.tensor.matmul(
                            ps1[:],
                            wt_sb[:, jc, t, it, :],
                            mf_sb[:, it, :],
                            start=(it == 0), stop=(it == NIT - 1),
                        )
                    nc.vector.tensor_scalar(k1_sb[:, jc, t, :], ps1[:],
                                            bt_sb[:, jc:jc + 1], 0.0,
                                            ALU.add, ALU.max)
                nc.sync.dma_start(kb[jc][:], k1_sb[:, jc].rearrange("p t n -> p (t n)"))
                nc.gpsimd.collective_compute(
                    "AllGather", ALU.bypass,
                    replica_groups=[list(range(N_CORES))],
                    ins=[kb[jc].opt()], outs=[kg[jc].opt()],
                )

            # gathered reload: global i-tile (2g+h) = kg[h] rows 128g..128(g+1)
            k1f = sbuf.tile([128, 2, N_CORES, 3 * NM], BF16, tag="k1f")
            for h in range(2):
                nc.sync.dma_start(
                    k1f[:, h],
                    kg[h].rearrange("(g p) f -> p g f", p=128),
                )

            # ---------------- CosFace precompute (fills the AllGather gap) ----
            # Only needs w_cls / iota / label; runs on PE/ACT/DVE while the
            # k1 AllGather halves are in flight.
            wsq_sb = sbuf.tile([128, 2, NM], F32, tag="wsq")
            nc.scalar.activation(wsq_sb[0:128, 0, :], wcls_sb[0:128, 0, :], AF.Square)
            nc.scalar.activation(wsq_sb[0:72, 1, :], wcls_sb[0:72, 1, :], AF.Square)
            wnorm_sb = sbuf.tile([128, 2], F32, tag="wnorm")
            wsA = psum.tile([128, 1], F32, tag="ep", name="wsA")
            nc.tensor.matmul(wsA[:], wsq_sb[0:128, 0, 0:128], onesc[0:128, :], start=True, stop=False)
            nc.tensor.matmul(wsA[:], wsq_sb[0:72, 1, 0:128], onesc[0:72, :], start=False, stop=True)
            wsB = psum.tile([72, 1], F32, tag="ep", name="wsB")
            nc.tensor.matmul(wsB[:], wsq_sb[0:128, 0, 128:NM], onesc[0:128, :], start=True, stop=False)
            nc.tensor.matmul(wsB[:], wsq_sb[0:72, 1, 128:NM], onesc[0:72, :], start=False, stop=True)
            nc.scalar.activation(wnorm_sb[:, 0:1], wsA[:], AF.Sqrt)
            nc.scalar.activation(wnorm_sb[0:72, 1:2], wsB[:], AF.Sqrt)
            winv_sb = sbuf.tile([128, 2], F32, tag="winv")
            nc.vector.reciprocal(winv_sb[:, 0:1], wnorm_sb[:, 0:1])
            nc.vector.reciprocal(winv_sb[0:72, 1:2], wnorm_sb[0:72, 1:2])
            winvrow_sb = sbuf.tile([1, NM], F32, tag="winvrow")
            wr1 = psum.tile([1, 128], F32, tag="ep", name="wr1")
            nc.tensor.transpose(wr1[:], winv_sb[:, 0:1], idn[:])
            nc.vector.tensor_copy(winvrow_sb[:, 0:128], wr1[:])
            wr2 = psum.tile([1, 72], F32, tag="ep", name="wr2")
            nc.tensor.transpose(wr2[:], winv_sb[0:72, 1:2], idn[0:72, 0:72])
            nc.vector.tensor_copy(winvrow_sb[:, 128:NM], wr2[:])
            wbps = psum.tile([RB, NM], F32, tag="ep", name="wbps")
            nc.tensor.matmul(wbps[:], ones1[:], winvrow_sb[:], start=True, stop=True)
            winvbS_sb = sbuf.tile([RB, NM], F32, tag="winvbS")
            nc.scalar.mul(winvbS_sb[:], wbps[:], S_SCALE)          # S/|w_c| broadcast
            maskSM_sb = sbuf.tile([RB, NM], F32, tag="maskSM")
            nc.vector.tensor_scalar(maskSM_sb[:], iota_sb[:], lbl_sb[:], None, ALU.is_equal)
            nc.vector.tensor_scalar(maskSM_sb[:], maskSM_sb[:], S_SCALE * M_MARGIN, None, ALU.mult)

            # ---------------- step 2: k2'[o,(s),n], o-sharded ----------------
            k2_sb = sbuf.tile([128, 2, 3, NM], BF16, tag="k2")
            for oc in range(2):
                psA = psum.tile([128, 2 * NM], F32, tag="ps2A", bufs=2, name=f"ps2A_{oc}")
                psB = psum.tile([128, NM], F32, tag="ps2B", bufs=1, name=f"ps2B_{oc}")
                n_it = 0
                for h in range(2):          # even i-tiles (half 0) first
                    for g in range(N_CORES):
                        slot = 8 * h + g    # wcT_sb host-ordered: even its first
                        first = (n_it == 0)
                        last = (n_it == 2 * N_CORES - 1)
                        kv = k1f[:, h, g]
                        l0 = wcT_sb[:, slot, 0, oc * 128:(oc + 1) * 128]
                        l1 = wcT_sb[:, slot, 1, oc * 128:(oc + 1) * 128]
                        l2 = wcT_sb[:, slot, 2, oc * 128:(oc + 1) * 128]
                        # dt=1: t'=0,1 -> s=0,1 (A[0:400])
                        nc.tensor.matmul(psA[:, 0:2 * NM], l1, kv[0:128, 0:2 * NM],
                                         start=first, stop=False)
                        # dt=0: t'=0 -> s=1 (A[200:400])
                        nc.tensor.matmul(psA[:, NM:2 * NM], l0, kv[0:128, 0:NM],
                                         start=False, stop=False)
                        # dt=2: t'=1,2 -> s=0,1 (A[0:400])
                        nc.tensor.matmul(psA[:, 0:2 * NM], l2, kv[0:128, NM:3 * NM],
                                         start=False, stop=last)
                        # dt=0: t'=1 -> s=2 (B)
                        nc.tensor.matmul(psB[:], l0, kv[0:128, NM:2 * NM],
                                         start=first, stop=False)
                        # dt=1: t'=2 -> s=2 (B)
                        nc.tensor.matmul(psB[:], l1, kv[0:128, 2 * NM:3 * NM],
                                         start=False, stop=last)
                        n_it += 1
                nc.vector.tensor_scalar(k2_sb[:, oc, 0, :], psA[:, 0:NM],
                                        bc_sb[:, oc:oc + 1], 0.0, ALU.add, ALU.max)
                nc.vector.tensor_scalar(k2_sb[:, oc, 1, :], psA[:, NM:2 * NM],
                                        bc_sb[:, oc:oc + 1], 0.0, ALU.add, ALU.max)
                nc.vector.tensor_scalar(k2_sb[:, oc, 2, :], psB[:],
                                        bc_sb[:, oc:oc + 1], 0.0, ALU.add, ALU.max)

            # ---------------- G: featT-stationary matmuls ----------------
            # out[x-chunk, t] = sum_u featT[u, x] V[u, t]; 42 chunks per PSUM
            # bank, one DVE cast-copy per bank. gbuf16 free index = 3*q + t,
            # q = chunk = b*2 + h.
            gbuf16 = sbuf.tile([128, NQ * 3], BF16, tag="gbuf16")
            CPB = 42
            nbanks = (NQ + CPB - 1) // CPB
            for bank in range(nbanks):
                c0 = bank * CPB
                c1 = min(c0 + CPB, NQ)
                gpk = psum.tile([128, CPB * 3], F32, tag="gpk", bufs=2, name=f"gpk{bank}")
                for q in range(c0, c1):
                    col = (q - c0) * 3
                    nc.tensor.matmul(gpk[:, col:col + 3],
                                     ft0_sb[:, q * 128:(q + 1) * 128],
                                     v_sb[0:128, 0, :], start=True, stop=False)
                    nc.tensor.matmul(gpk[:, col:col + 3],
                                     ft1_sb[0:68, q * 128:(q + 1) * 128],
                                     v_sb[0:68, 1, :], start=False, stop=True)
                nc.vector.tensor_copy(gbuf16[:, c0 * 3:c1 * 3], gpk[:, 0:(c1 - c0) * 3])

            # ---------------- cls partial: [64, 200] ----------------
            cps = psum.tile([BS, NM], F32, tag="ep", name="cps")
            first = True
            for h in range(2):
                for t in range(3):
                    lhs = gbuf16[:, 3 * h + t::6]
                    nc.tensor.matmul(cps[:], lhs[:, 0:BS], k2_sb[:, h, t, :],
                                     start=first, stop=(h == 1 and t == 2))
                    first = False
            clsp_sb = sbuf.tile([BS, NM], F32, tag="clsp")
            nc.vector.tensor_scalar(clsp_sb[:], cps[:], bsp_sb[:], None, ALU.add)

            # ---------------- ReduceScatter cls (core c keeps rows 8c..8c+8) ----
            cls_bounce = dram.tile([BS, NM], F32, name="cls_bounce")
            cls_red = dram.tile([RB, NM], F32, name="cls_red")
            nc.sync.dma_start(cls_bounce[:], clsp_sb[:])
            nc.gpsimd.collective_compute(
                "ReduceScatter", ALU.add,
                replica_groups=[list(range(N_CORES))],
                ins=[cls_bounce.opt()], outs=[cls_red.opt()],
            )
            cls_sb = sbuf.tile([RB, NM], F32, tag="cls")
            nc.sync.dma_start(cls_sb[:], cls_red[:])

            # ---- post-ReduceScatter chain (b_sp already folded in pre-scatter) ----
            sq_sb = sbuf.tile([RB, NM], F32, tag="sq")
            ss_sb = sbuf.tile([RB, 1], F32, tag="ss")
            nc.scalar.activation(sq_sb[:], cls_sb[:], AF.Square, accum_out=ss_sb[:])
            rt_sb = sbuf.tile([RB, 1], F32, tag="rt")
            nc.scalar.activation(rt_sb[:], ss_sb[:], AF.Sqrt)
            invx_sb = sbuf.tile([RB, 1], F32, tag="invx")
            nc.vector.reciprocal(invx_sb[:], rt_sb[:])
            clsT_sb = sbuf.tile([128, 2, RB], F32, tag="clsT")
            tp1 = psum.tile([128, RB], F32, tag="ep", name="tp1")
            nc.tensor.transpose(tp1[:], cls_sb[:, 0:128], idn[0:RB, 0:RB])
            nc.vector.tensor_copy(clsT_sb[0:128, 0, :], tp1[:])
            tp2 = psum.tile([72, RB], F32, tag="ep", name="tp2")
            nc.tensor.transpose(tp2[:], cls_sb[:, 128:NM], idn[0:RB, 0:RB])
            nc.vector.tensor_copy(clsT_sb[0:72, 1, :], tp2[:])
            cos_ps = psum.tile([RB, NM], F32, tag="ep", name="cos_ps")
            nc.tensor.matmul(cos_ps[:], clsT_sb[0:128, 0, :], wcls_sb[0:128, 0, :],
                             start=True, stop=False)
            nc.tensor.matmul(cos_ps[:], clsT_sb[0:72, 1, :], wcls_sb[0:72, 1, :],
                             start=False, stop=True)
            t1_sb = sbuf.tile([RB, NM], F32, tag="t1")
            nc.vector.scalar_tensor_tensor(t1_sb[:], cos_ps[:], invx_sb[:],
                                           winvbS_sb[:], ALU.mult, ALU.mult)
            out_sb = sbuf.tile([RB, NM], F32, tag="out")
            nc.vector.tensor_tensor(out_sb[:], t1_sb[:], maskSM_sb[:], ALU.subtract)
            nc.sync.dma_start(Y[:], out_sb[:])

    nc.compile()
    return nc


def _prep_inputs(feat, label, mem_feat, wt, bt, wc, bc, w_sp, b_sp, w_cls):
    bf = ml_dtypes.bfloat16
    f8 = ml_dtypes.float8_e4m3fn
    f32 = np.float32
    feat = np.ascontiguousarray(np.asarray(feat, dtype=f32))
    mem_feat = np.asarray(mem_feat, dtype=f32)
    wt = np.asarray(wt, dtype=f32)
    bt = np.asarray(bt, dtype=f32)
    wc = np.asarray(wc, dtype=f32)
    bc = np.asarray(bc, dtype=f32)
    w_sp = np.asarray(w_sp, dtype=f32)
    b_sp = np.asarray(b_sp, dtype=f32)
    w_cls = np.asarray(w_cls, dtype=f32)
    label = np.asarray(label)

    V = np.zeros((HW, 3), f32)
    V[:HW - 1, 0] = w_sp[0, 1:]
    V[:, 1] = w_sp[0, :]
    V[1:, 2] = w_sp[0, :HW - 1]
    vm = (V * SV).astype(bf)

    # mem_feat.T tiled [p, it, n]
    mft = np.ascontiguousarray(
        mem_feat.T.reshape(NIT, 128, NM).transpose(1, 0, 2)).reshape(128, NIT * NM).astype(bf)
    wclsT = np.ascontiguousarray(w_cls.T)                          # [200, 200] f32
    bsp = np.full((BS, 1), b_sp[0] / N_CORES * (SWT * SWC * SV), f32)
    lbl_full = label.astype(f32).reshape(BS, 1)
    iota = np.broadcast_to(np.arange(NM, dtype=f32), (RB, NM)).copy()
    IT_ORDER = list(range(0, NIT, 2)) + list(range(1, NIT, 2))     # even tiles first

    fv = feat.reshape(BS, C, HW)
    in_maps = []
    for c in range(N_CORES):
        J = slice(c * SH, (c + 1) * SH)
        # wt [p, jc, t, it, j] (scaled)
        wt_c = (wt[:, J, :] * SWT).reshape(NIT, 128, 2, 128, 3).transpose(
            1, 2, 4, 0, 3)                                          # [128,2,3,16,128]
        wt_c = np.ascontiguousarray(wt_c).reshape(128, 2 * 3 * NIT * 128).astype(bf)
        # wc.T [p, it(even-first), dt, o] (scaled)
        wcT_c = (wc[J, :, :] * SWC).transpose(1, 2, 0).reshape(NIT, 128, 3, SH)
        wcT_c = wcT_c[IT_ORDER].transpose(1, 0, 2, 3)               # [128,16,3,256]
        wcT_c = np.ascontiguousarray(wcT_c).reshape(128, NIT * 3 * SH).astype(bf)
        # feat slice [u, b*256+i] fp8
        ft_c = np.ascontiguousarray(
            fv[:, J, :].transpose(2, 0, 1)).reshape(HW, NX).astype(f8)
        bt_c = np.ascontiguousarray(
            (bt[J] * SWT).reshape(2, 128).T)                        # [128,2]
        bc_c = np.ascontiguousarray(
            (bc[J] * (SWT * SWC)).reshape(2, 128).T)                # [128,2]
        in_maps.append({
            "mft": mft, "wts": wt_c, "wct": wcT_c, "ftq": ft_c, "vm": vm,
            "btc": bt_c, "bcc": bc_c, "wclsT": wclsT,
            "bsp": bsp, "lbl": lbl_full[c * RB:(c + 1) * RB], "iota": iota,
        })
    return in_maps


def kernel(**inputs) -> np.ndarray:
    global LAST_RESULT
    if "nc" not in _CACHE:
        _CACHE["nc"] = build_nc()
    nc = _CACHE["nc"]
    in_maps = _prep_inputs(**inputs)
    try:
        res = bass_utils.run_bass_kernel_spmd(
            nc, in_maps, core_ids=list(range(N_CORES)),
            trace=TRACE, **TRACE_KW,
        )
    except Exception:
        # transient NRT/device hiccups recover on retry
        res = bass_utils.run_bass_kernel_spmd(
            nc, in_maps, core_ids=list(range(N_CORES)),
            trace=TRACE, **TRACE_KW,
        )
    LAST_RESULT = res
    return np.concatenate(
        [np.asarray(res.results[c]["y"], dtype=np.float32) for c in range(N_CORES)],
        axis=0,
    )


# revision 12
# speedup vs baseline: 1.3044x; 1.3044x over previous
"""Trainium2 Bass kernel for nn_DimCosSoftmaxModule (8-core SPMD).

Math (exact refactor of the reference):
  k1[n,j,t] = relu(sum_i mem_feat[n,i] wt[i,j,t] + bt[j])                 [200,2048,3]
  k2[n,o,s] = relu(sum_{i,dt} wc[o,i,dt] k1pad[n,i,s+dt-1] + bc[o])      [200,2048,3]
  conv/sp_down fold: cls[b,n] = sum_{i,t} G[b,i,t] k2[n,i,t] + b_sp
      where G[b,i,t] = sum_u feat[b,i,u] V[u,t],  V = shifted copies of w_sp
  out = 30*(cosine(cls, w_cls) - 0.5*onehot(label))

Sharding: tensor-parallel over the 2048 channel dim (256 ch/core).
  step1 column-sharded -> 2 pipelined AllGathers (one per 128-wide jc half)
  -> step2 o-sharded -> partial cls -> ReduceScatter (core c keeps batch
  rows 8c..8c+8) -> row-local CosFace.  Host reassembles the 8 row-shards.

Perf structure (vs the 185us baseline):
  * all input DMAs are contiguous per partition (host pre-layout), split
    across the three DMA rails (SP / ACT HWDGE + Pool SWDGE)
  * feat is stored fp8(e4m3) in DRAM and cast to bf16 during the SWDGE
    DMA itself; all PE math stays bf16 (sim rel-err 1.27e-2 < 2e-2)
  * scale folding keeps everything in fp8/bf16 range: wt*32, wc*32, V*16;
    the cosine normalization cancels the net cls scale (b_sp scaled too)
  * the AllGather is split into two halves pipelined behind step1, and
    step2 consumes the halves in order; G runs after step2 (feat lands
    last), keeping the PE continuously busy (p-state stays hot)
"""
import numpy as np
import ml_dtypes

import concourse.bass as bass
import concourse.bacc as bacc
import concourse.mybir as mybir
import concourse.tile as tile
from concourse import bass_utils
from concourse.masks import make_identity

N_CORES = 8
BS, C, HW = 64, 2048, 196
NM = 200                 # N_MEM == NUM_CLASSES
SH = C // N_CORES        # 256 channels per core
NIT = C // 128           # 16 i-tiles of 128
RB = BS // N_CORES       # 8 batch rows per core after reduce-scatter
S_SCALE, M_MARGIN = 30.0, 0.5
SWT, SWC, SV = 32.0, 32.0, 16.0     # fold scales (cancel in cosine)

BF16 = mybir.dt.bfloat16
F32 = mybir.dt.float32
FP8 = mybir.dt.float8e4
AF = mybir.ActivationFunctionType
ALU = mybir.AluOpType

TRACE = False
TRACE_KW = {}
LAST_RESULT = None
_CACHE = {}

NX = BS * SH          # 16384 G columns per core
NQ = NX // 128        # 128 x-chunks


def build_nc():
    nc = bacc.Bacc("TRN2", target_bir_lowering=False, debug=False, num_devices=N_CORES)

    # per-core external inputs (same shapes on every core, different data)
    MFT = nc.dram_tensor("mft", [128, NIT * NM], BF16, kind="ExternalInput")
    WTS = nc.dram_tensor("wts", [128, 2 * 3 * NIT * 128], BF16, kind="ExternalInput")
    WCT = nc.dram_tensor("wct", [128, NIT * 3 * SH], BF16, kind="ExternalInput")
    FTQ = nc.dram_tensor("ftq", [HW, NX], FP8, kind="ExternalInput")
    VM = nc.dram_tensor("vm", [HW, 3], BF16, kind="ExternalInput")
    BT = nc.dram_tensor("btc", [128, 2], F32, kind="ExternalInput")
    BC = nc.dram_tensor("bcc", [128, 2], F32, kind="ExternalInput")
    WCLS = nc.dram_tensor("wclsT", [NM, NM], F32, kind="ExternalInput")
    BSP = nc.dram_tensor("bsp", [BS, 1], F32, kind="ExternalInput")
    LBL = nc.dram_tensor("lbl", [RB, 1], F32, kind="ExternalInput")
    IOTA = nc.dram_tensor("iota", [RB, NM], F32, kind="ExternalInput")
    Y = nc.dram_tensor("y", [RB, NM], F32, kind="ExternalOutput")

    with tile.TileContext(nc) as tc:
        with (
            tc.tile_pool(name="sbuf", bufs=1) as sbuf,
            tc.tile_pool(name="psum", bufs=1, space="PSUM") as psum,
            tc.tile_pool(name="dram", bufs=1, space="DRAM") as dram,
        ):
            # ---------------- input DMAs: 3 rails, priority order ----------------
            # SP rail: wt halves (step1 weights), then dynamic traffic later.
            wt_sb = sbuf.tile([128, 2, 3, NIT, 128], BF16, tag="wts")
            wtv = WTS.rearrange("p (jc r) -> p jc r", jc=2)
            nc.sync.dma_start(wt_sb[:, 0].rearrange("p t it j -> p (t it j)"), wtv[:, 0])
            nc.sync.dma_start(wt_sb[:, 1].rearrange("p t it j -> p (t it j)"), wtv[:, 1])

            # ACT rail: mem_feat first (step1 rhs), smalls, then step2 weights.
            mf_sb = sbuf.tile([128, NIT, NM], BF16, tag="mf")
            nc.scalar.dma_start(mf_sb.rearrange("p a b -> p (a b)"), MFT[:, :])
            bt_sb = sbuf.tile([128, 2], F32, tag="bt")
            nc.scalar.dma_start(bt_sb[:], BT[:])
            bc_sb = sbuf.tile([128, 2], F32, tag="bc")
            nc.scalar.dma_start(bc_sb[:], BC[:])
            v_sb = sbuf.tile([128, 2, 3], BF16, tag="v")
            nc.scalar.dma_start(v_sb[0:128, 0, :], VM[0:128, :])
            nc.scalar.dma_start(v_sb[0:68, 1, :], VM[128:HW, :])
            wcls_sb = sbuf.tile([128, 2, NM], F32, tag="wcls")
            nc.scalar.dma_start(wcls_sb[0:128, 0, :], WCLS[0:128, :])
            nc.scalar.dma_start(wcls_sb[0:72, 1, :], WCLS[128:NM, :])
            bsp_sb = sbuf.tile([BS, 1], F32, tag="bsp")
            nc.scalar.dma_start(bsp_sb[:], BSP[:])
            lbl_sb = sbuf.tile([RB, 1], F32, tag="lbl")
            nc.scalar.dma_start(lbl_sb[:], LBL[:])
            iota_sb = sbuf.tile([RB, NM], F32, tag="iota")
            nc.scalar.dma_start(iota_sb[:], IOTA[:])
            wcT_sb = sbuf.tile([128, NIT, 3, SH], BF16, tag="wcT")
            wcv = WCT.rearrange("p (hh r) -> p hh r", hh=2)
            nc.scalar.dma_start(
                wcT_sb[:, 0:8].rearrange("p it t j -> p (it t j)"), wcv[:, 0])
            nc.scalar.dma_start(
                wcT_sb[:, 8:16].rearrange("p it t j -> p (it t j)"), wcv[:, 1])

            # feat: raw fp8 over the ACT rail; DVE casts to bf16 later (the
            # SWDGE cast path uses only 4 SDMA engines and its descriptor
            # generation monopolizes GpSimd, delaying collective doorbells).
            ft8_0 = sbuf.tile([128, NX], FP8, tag="ft80")
            ft8_1 = sbuf.tile([68, NX], FP8, tag="ft81")
            nc.scalar.dma_start(ft8_0[:], FTQ[0:128, :])
            nc.scalar.dma_start(ft8_1[:], FTQ[128:HW, :])
            ft0_sb = sbuf.tile([128, NX], BF16, tag="ft0")
            ft1_sb = sbuf.tile([68, NX], BF16, tag="ft1")

            # ---------------- constants ----------------
            idn = sbuf.tile([128, 128], F32, tag="idn")
            make_identity(nc, idn[:])
            ones1 = sbuf.tile([1, RB], F32, tag="ones1")
            nc.vector.memset(ones1[:], 1.0)
            onesc = sbuf.tile([128, 1], F32, tag="onesc")
            nc.vector.memset(onesc[:], 1.0)

            # ---------------- step 1 + pipelined AllGather halves ----------------
            # k1'[j,t,n] = relu(SWT * (mem_feat wt + bt)) for this core's 256 j.
            k1_sb = sbuf.tile([128, 2, 3, NM], BF16, tag="k1")
            kb = [dram.tile([128, 3 * NM], BF16, name=f"k1_bounce{h}") for h in range(2)]
            kg = [dram.tile([N_CORES * 128, 3 * NM], BF16, name=f"k1_gath{h}")
                  for h in range(2)]
            for jc in range(2):
                for t in range(3):
                    ps1 = psum.tile([128, NM], F32, tag="ps1", bufs=2,
                                    name=f"ps1_{jc}_{t}")
                    for it in range(NIT):
                        nc.tensor.matmul(
                            ps1[:],
                            wt_sb[:, jc, t, it, :],
                            mf_sb[:, it, :],
                            start=(it == 0), stop=(it == NIT - 1),
                        )
                    nc.vector.tensor_scalar(k1_sb[:, jc, t, :], ps1[:],
                                            bt_sb[:, jc:jc + 1], 0.0,
                                            ALU.add, ALU.max)
                nc.sync.dma_start(kb[jc][:], k1_sb[:, jc].rearrange("p t n -> p (t n)"))
                nc.gpsimd.collective_compute(
                    "AllGather", ALU.bypass,
                    replica_groups=[list(range(N_CORES))],
                    ins=[kb[jc].opt()], outs=[kg[jc].opt()],
                )

            # gathered reload: global i-tile (2g+h) = kg[h] rows 128g..128(g+1).
            # Quarter tiles so step2 can start on the first 4 tiles of a half.
            k1q = [[sbuf.tile([128, 4, 3 * NM], BF16, tag=f"k1q{h}{qq}",
                              name=f"k1q{h}{qq}") for qq in range(2)]
                   for h in range(2)]
            for h in range(2):
                kgv = kg[h].rearrange("(g p) f -> p g f", p=128)
                nc.sync.dma_start(k1q[h][0][:], kgv[:, 0:4])
                nc.sync.dma_start(k1q[h][1][:], kgv[:, 4:8])

            # ---------------- CosFace precompute (fills the AllGather gap) ----
            # Only needs w_cls / iota / label; runs on PE/ACT/DVE while the
            # k1 AllGather halves are in flight.
            wsq_sb = sbuf.tile([128, 2, NM], F32, tag="wsq")
            nc.scalar.activation(wsq_sb[0:128, 0, :], wcls_sb[0:128, 0, :], AF.Square)
            nc.scalar.activation(wsq_sb[0:72, 1, :], wcls_sb[0:72, 1, :], AF.Square)
            wnorm_sb = sbuf.tile([128, 2], F32, tag="wnorm")
            wsA = psum.tile([128, 1], F32, tag="ep", name="wsA")
            nc.tensor.matmul(wsA[:], wsq_sb[0:128, 0, 0:128], onesc[0:128, :], start=True, stop=False)
            nc.tensor.matmul(wsA[:], wsq_sb[0:72, 1, 0:128], onesc[0:72, :], start=False, stop=True)
            wsB = psum.tile([72, 1], F32, tag="ep", name="wsB")
            nc.tensor.matmul(wsB[:], wsq_sb[0:128, 0, 128:NM], onesc[0:128, :], start=True, stop=False)
            nc.tensor.matmul(wsB[:], wsq_sb[0:72, 1, 128:NM], onesc[0:72, :], start=False, stop=True)
            nc.scalar.activation(wnorm_sb[:, 0:1], wsA[:], AF.Sqrt)
            nc.scalar.activation(wnorm_sb[0:72, 1:2], wsB[:], AF.Sqrt)
            winv_sb = sbuf.tile([128, 2], F32, tag="winv")
            nc.vector.reciprocal(winv_sb[:, 0:1], wnorm_sb[:, 0:1])
            nc.vector.reciprocal(winv_sb[0:72, 1:2], wnorm_sb[0:72, 1:2])
            winvrow_sb = sbuf.tile([1, NM], F32, tag="winvrow")
            wr1 = psum.tile([1, 128], F32, tag="ep", name="wr1")
            nc.tensor.transpose(wr1[:], winv_sb[:, 0:1], idn[:])
            nc.vector.tensor_copy(winvrow_sb[:, 0:128], wr1[:])
            wr2 = psum.tile([1, 72], F32, tag="ep", name="wr2")
            nc.tensor.transpose(wr2[:], winv_sb[0:72, 1:2], idn[0:72, 0:72])
            nc.vector.tensor_copy(winvrow_sb[:, 128:NM], wr2[:])
            wbps = psum.tile([RB, NM], F32, tag="ep", name="wbps")
            nc.tensor.matmul(wbps[:], ones1[:], winvrow_sb[:], start=True, stop=True)
            winvbS_sb = sbuf.tile([RB, NM], F32, tag="winvbS")
            nc.scalar.mul(winvbS_sb[:], wbps[:], S_SCALE)          # S/|w_c| broadcast
            maskSM_sb = sbuf.tile([RB, NM], F32, tag="maskSM")
            nc.vector.tensor_scalar(maskSM_sb[:], iota_sb[:], lbl_sb[:], None, ALU.is_equal)
            nc.vector.tensor_scalar(maskSM_sb[:], maskSM_sb[:], S_SCALE * M_MARGIN, None, ALU.mult)

            # ---------------- step 2: k2'[o,(s),n], o-sharded ----------------
            k2_sb = sbuf.tile([128, 2, 3, NM], BF16, tag="k2")
            for oc in range(2):
                psA = psum.tile([128, 2 * NM], F32, tag="ps2A", bufs=2, name=f"ps2A_{oc}")
                psB = psum.tile([128, NM], F32, tag="ps2B", bufs=1, name=f"ps2B_{oc}")
                n_it = 0
                for h in range(2):          # even i-tiles (half 0) first
                    for g in range(N_CORES):
                        slot = 8 * h + g    # wcT_sb host-ordered: even its first
                        first = (n_it == 0)
                        last = (n_it == 2 * N_CORES - 1)
                        kv = k1q[h][g // 4][:, g % 4]
                        l0 = wcT_sb[:, slot, 0, oc * 128:(oc + 1) * 128]
                        l1 = wcT_sb[:, slot, 1, oc * 128:(oc + 1) * 128]
                        l2 = wcT_sb[:, slot, 2, oc * 128:(oc + 1) * 128]
                        # dt=1: t'=0,1 -> s=0,1 (A[0:400])
                        nc.tensor.matmul(psA[:, 0:2 * NM], l1, kv[0:128, 0:2 * NM],
                                         start=first, stop=False)
                        # dt=0: t'=0 -> s=1 (A[200:400])
                        nc.tensor.matmul(psA[:, NM:2 * NM], l0, kv[0:128, 0:NM],
                                         start=False, stop=False)
                        # dt=2: t'=1,2 -> s=0,1 (A[0:400])
                        nc.tensor.matmul(psA[:, 0:2 * NM], l2, kv[0:128, NM:3 * NM],
                                         start=False, stop=last)
                        # dt=0: t'=1 -> s=2 (B)
                        nc.tensor.matmul(psB[:], l0, kv[0:128, NM:2 * NM],
                                         start=first, stop=False)
                        # dt=1: t'=2 -> s=2 (B)
                        nc.tensor.matmul(psB[:], l1, kv[0:128, 2 * NM:3 * NM],
                                         start=False, stop=last)
                        n_it += 1
                nc.vector.tensor_scalar(k2_sb[:, oc, 0, :], psA[:, 0:NM],
                                        bc_sb[:, oc:oc + 1], 0.0, ALU.add, ALU.max)
                nc.vector.tensor_scalar(k2_sb[:, oc, 1, :], psA[:, NM:2 * NM],
                                        bc_sb[:, oc:oc + 1], 0.0, ALU.add, ALU.max)
                nc.vector.tensor_scalar(k2_sb[:, oc, 2, :], psB[:],
                                        bc_sb[:, oc:oc + 1], 0.0, ALU.add, ALU.max)
                if oc == 0:
                    # DVE fp8->bf16 feat casts, slotted here so they overlap
                    # step2-oc1 matmuls and finish before G needs them.
                    CH = NX // 4
                    for a in range(4):
                        nc.vector.tensor_copy(ft0_sb[:, a * CH:(a + 1) * CH],
                                              ft8_0[:, a * CH:(a + 1) * CH])
                    for a in range(2):
                        nc.vector.tensor_copy(ft1_sb[:, a * 2 * CH:(a + 1) * 2 * CH],
                                              ft8_1[:, a * 2 * CH:(a + 1) * 2 * CH])

            # ---------------- G: featT-stationary matmuls ----------------
            # out[x-chunk, t] = sum_u featT[u, x] V[u, t]; 42 chunks per PSUM
            # bank, one DVE cast-copy per bank. gbuf16 free index = 3*q + t,
            # q = chunk = b*2 + h.
            gbuf16 = sbuf.tile([128, NQ * 3], BF16, tag="gbuf16")
            CPB = 42
            nbanks = (NQ + CPB - 1) // CPB
            for bank in range(nbanks):
                c0 = bank * CPB
                c1 = min(c0 + CPB, NQ)
                gpk = psum.tile([128, CPB * 3], F32, tag="gpk", bufs=2, name=f"gpk{bank}")
                for q in range(c0, c1):
                    col = (q - c0) * 3
                    nc.tensor.matmul(gpk[:, col:col + 3],
                                     ft0_sb[:, q * 128:(q + 1) * 128],
                                     v_sb[0:128, 0, :], start=True, stop=False)
                    nc.tensor.matmul(gpk[:, col:col + 3],
                                     ft1_sb[0:68, q * 128:(q + 1) * 128],
                                     v_sb[0:68, 1, :], start=False, stop=True)
                nc.vector.tensor_copy(gbuf16[:, c0 * 3:c1 * 3], gpk[:, 0:(c1 - c0) * 3])

            # ---------------- cls partial: [64, 200] ----------------
            cps = psum.tile([BS, NM], F32, tag="ep", name="cps")
            first = True
            for h in range(2):
                for t in range(3):
                    lhs = gbuf16[:, 3 * h + t::6]
                    nc.tensor.matmul(cps[:], lhs[:, 0:BS], k2_sb[:, h, t, :],
                                     start=first, stop=(h == 1 and t == 2))
                    first = False
            clsp_sb = sbuf.tile([BS, NM], F32, tag="clsp")
            nc.vector.tensor_scalar(clsp_sb[:], cps[:], bsp_sb[:], None, ALU.add)

            # ---------------- ReduceScatter cls (core c keeps rows 8c..8c+8) ----
            cls_bounce = dram.tile([BS, NM], F32, name="cls_bounce")
            cls_red = dram.tile([RB, NM], F32, name="cls_red")
            nc.sync.dma_start(cls_bounce[:], clsp_sb[:])
            nc.gpsimd.collective_compute(
                "ReduceScatter", ALU.add,
                replica_groups=[list(range(N_CORES))],
                ins=[cls_bounce.opt()], outs=[cls_red.opt()],
            )
            cls_sb = sbuf.tile([RB, NM], F32, tag="cls")
            nc.sync.dma_start(cls_sb[:], cls_red[:])

            # ---- post-ReduceScatter chain (b_sp already folded in pre-scatter) ----
            sq_sb = sbuf.tile([RB, NM], F32, tag="sq")
            ss_sb = sbuf.tile([RB, 1], F32, tag="ss")
            nc.scalar.activation(sq_sb[:], cls_sb[:], AF.Square, accum_out=ss_sb[:])
            rt_sb = sbuf.tile([RB, 1], F32, tag="rt")
            nc.scalar.activation(rt_sb[:], ss_sb[:], AF.Sqrt)
            invx_sb = sbuf.tile([RB, 1], F32, tag="invx")
            nc.vector.reciprocal(invx_sb[:], rt_sb[:])
            clsT_sb = sbuf.tile([128, 2, RB], F32, tag="clsT")
            tp1 = psum.tile([128, RB], F32, tag="ep", name="tp1")
            nc.tensor.transpose(tp1[:], cls_sb[:, 0:128], idn[0:RB, 0:RB])
            nc.vector.tensor_copy(clsT_sb[0:128, 0, :], tp1[:])
            tp2 = psum.tile([72, RB], F32, tag="ep", name="tp2")
            nc.tensor.transpose(tp2[:], cls_sb[:, 128:NM], idn[0:RB, 0:RB])
            nc.vector.tensor_copy(clsT_sb[0:72, 1, :], tp2[:])
            cos_ps = psum.tile([RB, NM], F32, tag="ep", name="cos_ps")
            nc.tensor.matmul(cos_ps[:], clsT_sb[0:128, 0, :], wcls_sb[0:128, 0, :],
                             start=True, stop=False)
            nc.tensor.matmul(cos_ps[:], clsT_sb[0:72, 1, :], wcls_sb[0:72, 1, :],
                             start=False, stop=True)
            t1_sb = sbuf.tile([RB, NM], F32, tag="t1")
            nc.vector.scalar_tensor_tensor(t1_sb[:], cos_ps[:], invx_sb[:],
                                           winvbS_sb[:], ALU.mult, ALU.mult)
            out_sb = sbuf.tile([RB, NM], F32, tag="out")
            nc.vector.tensor_tensor(out_sb[:], t1_sb[:], maskSM_sb[:], ALU.subtract)
            nc.sync.dma_start(Y[:], out_sb[:])

    nc.compile()
    return nc


def _prep_inputs(feat, label, mem_feat, wt, bt, wc, bc, w_sp, b_sp, w_cls):
    bf = ml_dtypes.bfloat16
    f8 = ml_dtypes.float8_e4m3fn
    f32 = np.float32
    feat = np.ascontiguousarray(np.asarray(feat, dtype=f32))
    mem_feat = np.asarray(mem_feat, dtype=f32)
    wt = np.asarray(wt, dtype=f32)
    bt = np.asarray(bt, dtype=f32)
    wc = np.asarray(wc, dtype=f32)
    bc = np.asarray(bc, dtype=f32)
    w_sp = np.asarray(w_sp, dtype=f32)
    b_sp = np.asarray(b_sp, dtype=f32)
    w_cls = np.asarray(w_cls, dtype=f32)
    label = np.asarray(label)

    V = np.zeros((HW, 3), f32)
    V[:HW - 1, 0] = w_sp[0, 1:]
    V[:, 1] = w_sp[0, :]
    V[1:, 2] = w_sp[0, :HW - 1]
    vm = (V * SV).astype(bf)

    # mem_feat.T tiled [p, it, n]
    mft = np.ascontiguousarray(
        mem_feat.T.reshape(NIT, 128, NM).transpose(1, 0, 2)).reshape(128, NIT * NM).astype(bf)
    wclsT = np.ascontiguousarray(w_cls.T)                          # [200, 200] f32
    bsp = np.full((BS, 1), b_sp[0] / N_CORES * (SWT * SWC * SV), f32)
    lbl_full = label.astype(f32).reshape(BS, 1)
    iota = np.broadcast_to(np.arange(NM, dtype=f32), (RB, NM)).copy()
    IT_ORDER = list(range(0, NIT, 2)) + list(range(1, NIT, 2))     # even tiles first

    fv = feat.reshape(BS, C, HW)
    in_maps = []
    for c in range(N_CORES):
        J = slice(c * SH, (c + 1) * SH)
        # wt [p, jc, t, it, j] (scaled)
        wt_c = (wt[:, J, :] * SWT).reshape(NIT, 128, 2, 128, 3).transpose(
            1, 2, 4, 0, 3)                                          # [128,2,3,16,128]
        wt_c = np.ascontiguousarray(wt_c).reshape(128, 2 * 3 * NIT * 128).astype(bf)
        # wc.T [p, it(even-first), dt, o] (scaled)
        wcT_c = (wc[J, :, :] * SWC).transpose(1, 2, 0).reshape(NIT, 128, 3, SH)
        wcT_c = wcT_c[IT_ORDER].transpose(1, 0, 2, 3)               # [128,16,3,256]
        wcT_c = np.ascontiguousarray(wcT_c).reshape(128, NIT * 3 * SH).astype(bf)
        # feat slice [u, b*256+i] fp8
        ft_c = np.ascontiguousarray(
            fv[:, J, :].transpose(2, 0, 1)).reshape(HW, NX).astype(f8)
        bt_c = np.ascontiguousarray(
            (bt[J] * SWT).reshape(2, 128).T)                        # [128,2]
        bc_c = np.ascontiguousarray(
            (bc[J] * (SWT * SWC)).reshape(2, 128).T)                # [128,2]
        in_maps.append({
            "mft": mft, "wts": wt_c, "wct": wcT_c, "ftq": ft_c, "vm": vm,
            "btc": bt_c, "bcc": bc_c, "wclsT": wclsT,
            "bsp": bsp, "lbl": lbl_full[c * RB:(c + 1) * RB], "iota": iota,
        })
    return in_maps


def kernel(**inputs) -> np.ndarray:
    global LAST_RESULT
    if "nc" not in _CACHE:
        _CACHE["nc"] = build_nc()
    nc = _CACHE["nc"]
    in_maps = _prep_inputs(**inputs)
    try:
        res = bass_utils.run_bass_kernel_spmd(
            nc, in_maps, core_ids=list(range(N_CORES)),
            trace=TRACE, **TRACE_KW,
        )
    except Exception:
        # transient NRT/device hiccups recover on retry
        res = bass_utils.run_bass_kernel_spmd(
            nc, in_maps, core_ids=list(range(N_CORES)),
            trace=TRACE, **TRACE_KW,
        )
    LAST_RESULT = res
    return np.concatenate(
        [np.asarray(res.results[c]["y"], dtype=np.float32) for c in range(N_CORES)],
        axis=0,
    )
